# revision 4
# baseline (speedup 1.0000x reference)
"""GroupedQueryAttention on 8 Trainium2 NeuronCores (Bass/Tile).

Tensor-parallel over heads: core c owns q-heads 4c..4c+3 and kv-head c.
Each core: bf16 projections + on-chip interleaved RoPE (pair-swap via a
permutation matmul), causal attention per 256-row q-block (softmax without
max-subtraction; denominator via a ones-column in the PV matmul), then an
AllGather of y^T and the per-core output-column slice of the wo projection.

Host side: shards/transposes weights, converts to bf16, builds RoPE tables
cos2/sin2 ([64, B*T], sign-folded), pair-swap permutation, causal masks.
"""
import os
import sys
import types

os.environ.setdefault("JAX_PLATFORMS", "cpu,axon")

import numpy as np
import ml_dtypes

BF = ml_dtypes.bfloat16

# Optional NTFF-profile hook injection (lets BASS_TRACE=1 capture exec_time).
try:
    import antenv.axon_hooks  # noqa: F401
except ImportError:
    try:
        _hm = types.ModuleType("antenv.axon_hooks")
        _hs = [None]
        _hm.set_axon_ntff_profile_hook = lambda h: _hs.__setitem__(0, h)
        _hm.get_axon_ntff_profile_hook = lambda: _hs[0]
        sys.modules["antenv.axon_hooks"] = _hm
        import antenv

        antenv.axon_hooks = _hm
        from trn_agent_boot.trn_boot import _ntff_profile_via_ctypes

        _hook = _ntff_profile_via_ctypes("/opt/axon/libaxon_pjrt.so")
        if _hook is not None:
            _hm.set_axon_ntff_profile_hook(_hook)
    except Exception:
        pass

import concourse.bass as bass
import concourse.tile as tile
from concourse import bacc, mybir
from concourse.bass_utils import run_bass_kernel_spmd

B, T, DIM = 2, 2048, 2048
N_HEADS, N_KV_HEADS, HEAD_DIM = 32, 8, 64
NCORES = 8
HPC = N_HEADS // NCORES  # 4 q heads per core
DT = mybir.dt.bfloat16
F32 = mybir.dt.float32

LAST_RESULTS = None  # BassKernelResults of the most recent run (for test.py)


def build_nc(Tt=T):
    """Build + compile the SPMD program (same for all 8 cores)."""
    BT = B * Tt
    PQB = Tt // 256  # q-blocks per batch
    NQB = BT // 256
    NCH = BT // 512  # all-gather chunks
    NTT = BT // 128  # 128-token tiles
    NGG = BT // 1024  # x streaming groups
    assert BT % 1024 == 0

    nc = bacc.Bacc("TRN2", target_bir_lowering=False, debug=False,
                   num_devices=NCORES)

    xT = nc.declare_dram_parameter("xT", [DIM, BT], DT, isOutput=False)
    wqT = nc.declare_dram_parameter("wqT", [DIM, 256], DT, isOutput=False)
    wkvT = nc.declare_dram_parameter("wkvT", [DIM, 128], DT, isOutput=False)
    woT = nc.declare_dram_parameter("woT", [DIM, 256], DT, isOutput=False)
    cos2 = nc.declare_dram_parameter("cos2", [128, BT], F32, isOutput=False)
    sin2 = nc.declare_dram_parameter("sin2", [128, BT], F32, isOutput=False)
    pswp = nc.declare_dram_parameter("pswp", [128, 128], DT, isOutput=False)
    ident = nc.declare_dram_parameter("ident", [64, 64], DT, isOutput=False)
    mask0 = nc.declare_dram_parameter("mask0", [128, 256], DT, isOutput=False)
    mask1 = nc.declare_dram_parameter("mask1", [128, 256], DT, isOutput=False)
    outp = nc.declare_dram_parameter("out", [BT, 256], F32, isOutput=True)

    with tile.TileContext(nc) as tc:
        with tc.tile_pool(name="persist", bufs=1) as persist, \
             tc.tile_pool(name="dram", bufs=1, space="DRAM") as dramp:
            qT_sb = persist.tile([64, HPC, BT], DT)
            kT_sb = persist.tile([64, BT], DT)
            v_sb = persist.tile([128, NTT, 65], DT)
            yloc = persist.tile([64, HPC, BT], DT)
            wo_sb = persist.tile([128, 16, 256], DT)
            pswp_sb = persist.tile([128, 128], DT)
            id_sb = persist.tile([64, 64], DT)
            m0_sb = persist.tile([128, 256], DT)
            m1_sb = persist.tile([128, 256], DT)

            ytloc = [dramp.tile([256, 512], DT, name=f"ytloc{c}")
                     for c in range(NCH)]
            ytful = [dramp.tile([2048, 512], DT, addr_space="Shared",
                                name=f"ytful{c}") for c in range(NCH)]

            nc.sync.dma_start(out=pswp_sb, in_=pswp[:])
            nc.sync.dma_start(out=id_sb, in_=ident[:])
            nc.sync.dma_start(out=m0_sb, in_=mask0[:])
            nc.sync.dma_start(out=m1_sb, in_=mask1[:])
            for dt_ in range(16):
                nc.sync.dma_start(out=wo_sb[:, dt_, :],
                                  in_=woT[128 * dt_:128 * dt_ + 128, :])
            nc.vector.memset(v_sb[:, :, 64:65], 1.0)

            # ---------------- phase 1: projections + RoPE ----------------
            with tc.tile_pool(name="wpool", bufs=1) as wpool, \
                 tc.tile_pool(name="xs", bufs=2) as xs, \
                 tc.tile_pool(name="cs", bufs=2) as cs, \
                 tc.tile_pool(name="ptmp", bufs=3) as ptmp, \
                 tc.tile_pool(name="ps_mm", bufs=2, space="PSUM") as ps_mm, \
                 tc.tile_pool(name="ps_sw", bufs=2, space="PSUM") as ps_sw, \
                 tc.tile_pool(name="ps_vt", bufs=2, space="PSUM") as ps_vt:
                wq_sb = wpool.tile([128, 16, 256], DT)
                wkv_sb = wpool.tile([128, 16, 128], DT)
                for ct in range(16):
                    nc.sync.dma_start(out=wq_sb[:, ct, :],
                                      in_=wqT[128 * ct:128 * ct + 128, :])
                    nc.sync.dma_start(out=wkv_sb[:, ct, :],
                                      in_=wkvT[128 * ct:128 * ct + 128, :])

                for gg in range(NGG):
                    xt = xs.tile([128, 16, 1024], DT, tag="xt")
                    for ct in range(16):
                        nc.sync.dma_start(
                            out=xt[:, ct, :],
                            in_=xT[128 * ct:128 * ct + 128,
                                   1024 * gg:1024 * gg + 1024])
                    cost = cs.tile([128, 1024], F32, tag="cost")
                    sint = cs.tile([128, 1024], F32, tag="sint")
                    nc.sync.dma_start(out=cost,
                                      in_=cos2[:, 1024 * gg:1024 * gg + 1024])
                    nc.sync.dma_start(out=sint,
                                      in_=sin2[:, 1024 * gg:1024 * gg + 1024])
                    for sub in range(2):
                        cols = slice(1024 * gg + 512 * sub,
                                     1024 * gg + 512 * sub + 512)
                        lc = slice(512 * sub, 512 * sub + 512)
                        # ---- kv projection ----
                        pkv = ps_mm.tile([128, 512], F32, tag="mm")
                        for ct in range(16):
                            nc.tensor.matmul(pkv, lhsT=wkv_sb[:, ct, :],
                                             rhs=xt[:, ct, lc],
                                             start=(ct == 0), stop=(ct == 15))
                        kraw = ptmp.tile([64, 512], DT, tag="kraw")
                        vraw = ptmp.tile([64, 512], DT, tag="vraw")
                        nc.vector.tensor_copy(kraw, pkv[0:64, :])
                        nc.vector.tensor_copy(vraw, pkv[64:128, :])
                        # k: pair-swap + rope -> kT_sb
                        pksw = ps_sw.tile([64, 512], F32, tag="sw")
                        nc.tensor.matmul(pksw, lhsT=pswp_sb[0:64, 0:64],
                                         rhs=kraw, start=True, stop=True)
                        t0k = ptmp.tile([64, 512], F32, tag="t0")
                        t1k = ptmp.tile([64, 512], F32, tag="t1")
                        nc.vector.tensor_mul(t0k, kraw, cost[0:64, lc])
                        nc.vector.tensor_mul(t1k, pksw, sint[0:64, lc])
                        nc.vector.tensor_add(kT_sb[:, cols], t0k, t1k)
                        # v: transpose to token-major [t, d] (+ ones col)
                        for t4 in range(4):
                            pvt = ps_vt.tile([128, 64], DT, tag="vt")
                            nc.tensor.transpose(
                                pvt, vraw[:, 128 * t4:128 * t4 + 128], id_sb)
                            TT = (1024 * gg + 512 * sub) // 128 + t4
                            nc.vector.tensor_copy(v_sb[:, TT, 0:64], pvt)
                        # ---- q projection (2 head-pair tiles) ----
                        for m in range(2):
                            pq = ps_mm.tile([128, 512], F32, tag="mm")
                            for ct in range(16):
                                nc.tensor.matmul(
                                    pq, lhsT=wq_sb[:, ct, 128 * m:128 * m + 128],
                                    rhs=xt[:, ct, lc],
                                    start=(ct == 0), stop=(ct == 15))
                            qraw = ptmp.tile([128, 512], DT, tag="qraw")
                            nc.vector.tensor_copy(qraw, pq)
                            pqsw = ps_sw.tile([128, 512], F32, tag="sw")
                            nc.tensor.matmul(pqsw, lhsT=pswp_sb, rhs=qraw,
                                             start=True, stop=True)
                            for hh in range(2):
                                h = 2 * m + hh
                                rows = slice(64 * hh, 64 * hh + 64)
                                t0 = ptmp.tile([64, 512], F32, tag="t0")
                                t1 = ptmp.tile([64, 512], F32, tag="t1")
                                nc.vector.tensor_mul(t0, qraw[rows, :],
                                                     cost[rows, lc])
                                nc.vector.tensor_mul(t1, pqsw[rows, :],
                                                     sint[rows, lc])
                                nc.vector.tensor_add(qT_sb[:, h, cols], t0, t1)

            # ------------- phase 2: attention + gather + out-proj -------------
            with tc.tile_pool(name="ps_att", bufs=2, space="PSUM") as ps_att, \
                 tc.tile_pool(name="ps_pv", bufs=1, space="PSUM") as ps_pv, \
                 tc.tile_pool(name="ps_out", bufs=1, space="PSUM") as ps_out, \
                 tc.tile_pool(name="ptp", bufs=3) as ptp, \
                 tc.tile_pool(name="normp", bufs=3) as normp, \
                 tc.tile_pool(name="ytfp", bufs=2) as ytfp, \
                 tc.tile_pool(name="osbp", bufs=2) as osbp:
                for ch in range(NCH):
                    for qbl in range(2):
                        qb = 2 * ch + qbl
                        b, p = qb // PQB, qb % PQB
                        qcols = slice(256 * qb, 256 * qb + 256)
                        nk = 2 * (p + 1)
                        for h in range(HPC):
                            po = ps_pv.tile([65, 256], F32, tag="pv")
                            for kb in range((nk + 5) // 6):
                                kt0 = 6 * kb
                                ktn = min(6, nk - kt0)
                                sm = ps_att.tile([128, 6, 256], F32,
                                                 tag="smega")
                                for i in range(ktn):
                                    kt = kt0 + i
                                    kc = slice(b * Tt + 128 * kt,
                                               b * Tt + 128 * kt + 128)
                                    nc.tensor.matmul(
                                        sm[:, i, :], lhsT=kT_sb[:, kc],
                                        rhs=qT_sb[:, h, qcols],
                                        start=True, stop=True)
                                pt = ptp.tile([128, 6, 256], DT, tag="pt")
                                nc.scalar.activation(
                                    pt[:, :ktn, :], sm[:, :ktn, :],
                                    mybir.ActivationFunctionType.Exp,
                                    scale=0.125)
                                for i in range(ktn):
                                    kt = kt0 + i
                                    if kt == nk - 2:
                                        nc.vector.tensor_mul(
                                            pt[:, i, :], pt[:, i, :], m0_sb)
                                    elif kt == nk - 1:
                                        nc.vector.tensor_mul(
                                            pt[:, i, :], pt[:, i, :], m1_sb)
                                for i in range(ktn):
                                    kt = kt0 + i
                                    ktg = b * (Tt // 128) + kt
                                    nc.tensor.matmul(
                                        po, lhsT=v_sb[:, ktg, :],
                                        rhs=pt[:, i, :],
                                        start=(kt == 0), stop=(kt == nk - 1))
                            rec = normp.tile([1, 256], F32, tag="rec")
                            nc.vector.reciprocal(rec, po[64:65, :])
                            bca = normp.tile([64, 256], F32, tag="bca")
                            nc.gpsimd.partition_broadcast(bca, rec)
                            nc.vector.tensor_mul(yloc[:, h, qcols],
                                                 po[0:64, :], bca)
                    # gather chunk ch across cores
                    ccols = slice(512 * ch, 512 * ch + 512)
                    nc.sync.dma_start(
                        out=ytloc[ch].rearrange("(h d) t -> d h t", h=HPC),
                        in_=yloc[:, :, ccols])
                    nc.gpsimd.collective_compute(
                        "AllGather", mybir.AluOpType.bypass,
                        replica_groups=[list(range(NCORES))],
                        ins=[ytloc[ch]], outs=[ytful[ch]])
                    # out-projection for this chunk
                    ytf = ytfp.tile([128, 16, 512], DT, tag="ytf")
                    for dt_ in range(16):
                        nc.sync.dma_start(
                            out=ytf[:, dt_, :],
                            in_=ytful[ch][128 * dt_:128 * dt_ + 128, :])
                    for tt in range(4):
                        pout = ps_out.tile([128, 256], F32, tag="out")
                        for dt_ in range(16):
                            nc.tensor.matmul(
                                pout,
                                lhsT=ytf[:, dt_, 128 * tt:128 * tt + 128],
                                rhs=wo_sb[:, dt_, :],
                                start=(dt_ == 0), stop=(dt_ == 15))
                        ot = osbp.tile([128, 256], F32, tag="osb")
                        nc.vector.tensor_copy(ot, pout)
                        r0 = 512 * ch + 128 * tt
                        nc.sync.dma_start(out=outp[r0:r0 + 128, :], in_=ot)

    nc.compile()
    return nc


def host_inputs(x, cos, sin, wq, wk, wv, wo, Tt=T):
    """Build the 8 per-core input maps from full fp32 inputs."""
    BT = B * Tt
    x = np.asarray(x, np.float32)[:, :Tt, :]
    xT = np.ascontiguousarray(x.reshape(BT, DIM).T).astype(BF)

    cos = np.asarray(cos, np.float32)[:Tt]
    sin = np.asarray(sin, np.float32)[:Tt]
    cos2 = np.empty((128, BT), np.float32)
    sin2 = np.empty((128, BT), np.float32)
    for d in range(128):
        j = (d % 64) // 2
        cos2[d] = np.tile(cos[:, j], B)
        sin2[d] = np.tile(sin[:, j] if d % 2 else -sin[:, j], B)

    pswp = np.zeros((128, 128), BF)
    for i in range(128):
        pswp[i, i ^ 1] = 1
    ident = np.eye(64, dtype=BF)
    ii = np.arange(128)[:, None]
    jj = np.arange(256)[None, :]
    mask0 = (jj >= ii).astype(BF)
    mask1 = (jj >= ii + 128).astype(BF)

    maps = []
    for c in range(NCORES):
        qs = slice(256 * c, 256 * c + 256)
        ks = slice(64 * c, 64 * c + 64)
        wkv = np.concatenate([wk[ks], wv[ks]], axis=0)
        maps.append({
            "xT": xT,
            "wqT": np.ascontiguousarray(wq[qs].T).astype(BF),
            "wkvT": np.ascontiguousarray(wkv.T).astype(BF),
            "woT": np.ascontiguousarray(wo[qs].T).astype(BF),
            "cos2": cos2, "sin2": sin2,
            "pswp": pswp, "ident": ident,
            "mask0": mask0, "mask1": mask1,
        })
    return maps


_NC_CACHE = {}


def _get_nc(Tt=T):
    if Tt not in _NC_CACHE:
        _NC_CACHE[Tt] = build_nc(Tt)
    return _NC_CACHE[Tt]


def kernel(x, cos, sin, wq, wk, wv, wo):
    global LAST_RESULTS
    nc = _get_nc(T)
    maps = host_inputs(x, cos, sin, wq, wk, wv, wo)
    res = run_bass_kernel_spmd(nc, maps, core_ids=list(range(NCORES)))
    LAST_RESULTS = res
    out = np.empty((B * T, DIM), np.float32)
    for c in range(NCORES):
        out[:, 256 * c:256 * c + 256] = res.results[c]["out"]
    return out.reshape(B, T, DIM)


# revision 6
# speedup vs baseline: 1.1006x; 1.1006x over previous
"""GroupedQueryAttention on 8 Trainium2 NeuronCores (Bass/Tile).

Tensor-parallel over heads: core c owns q-heads 4c..4c+3 and kv-head c.
Per core: bf16 projections + on-chip interleaved RoPE (pair-swap via a
permutation matmul), causal attention per 256-row q-block (softmax without
max-subtraction; denominator via a ones-column in the PV matmul), then an
AllGather of y^T and a transposed out-projection producing the core's
256-column slice of the output (host re-transposes and concatenates).

Attention matmuls process head-pairs (N=512 moving operand) and share the
stationary k/v tiles; softmax exp runs on ScalarE in [128,4,256] batches.
"""
import os
import sys
import types

os.environ.setdefault("JAX_PLATFORMS", "cpu,axon")

import numpy as np
import ml_dtypes

BF = ml_dtypes.bfloat16

# Optional NTFF-profile hook injection (lets BASS_TRACE=1 capture exec_time).
try:
    import antenv.axon_hooks  # noqa: F401
except ImportError:
    try:
        _hm = types.ModuleType("antenv.axon_hooks")
        _hs = [None]
        _hm.set_axon_ntff_profile_hook = lambda h: _hs.__setitem__(0, h)
        _hm.get_axon_ntff_profile_hook = lambda: _hs[0]
        sys.modules["antenv.axon_hooks"] = _hm
        import antenv

        antenv.axon_hooks = _hm
        from trn_agent_boot.trn_boot import _ntff_profile_via_ctypes

        _hook = _ntff_profile_via_ctypes("/opt/axon/libaxon_pjrt.so")
        if _hook is not None:
            _hm.set_axon_ntff_profile_hook(_hook)
    except Exception:
        pass

import concourse.bass as bass
import concourse.tile as tile
from concourse import bacc, mybir
from concourse.bass_utils import run_bass_kernel_spmd

B, T, DIM = 2, 2048, 2048
N_HEADS, N_KV_HEADS, HEAD_DIM = 32, 8, 64
NCORES = 8
HPC = N_HEADS // NCORES  # 4 q heads per core
DT = mybir.dt.bfloat16
F32 = mybir.dt.float32

LAST_RESULTS = None  # BassKernelResults of the most recent run (for test.py)


def build_nc(Tt=T):
    """Build + compile the SPMD program (same for all 8 cores)."""
    BT = B * Tt
    PQB = Tt // 256  # q-blocks per batch
    NCH = BT // 512  # all-gather chunks
    NTT = BT // 128  # 128-token tiles
    NGG = BT // 1024  # x streaming groups
    assert BT % 1024 == 0

    nc = bacc.Bacc("TRN2", target_bir_lowering=False, debug=False,
                   num_devices=NCORES)

    xT = nc.declare_dram_parameter("xT", [DIM, BT], DT, isOutput=False)
    wqT = nc.declare_dram_parameter("wqT", [DIM, 256], DT, isOutput=False)
    wkvT = nc.declare_dram_parameter("wkvT", [DIM, 128], DT, isOutput=False)
    woT = nc.declare_dram_parameter("woT", [DIM, 256], DT, isOutput=False)
    cos2 = nc.declare_dram_parameter("cos2", [128, BT], DT, isOutput=False)
    sin2 = nc.declare_dram_parameter("sin2", [128, BT], DT, isOutput=False)
    pswp = nc.declare_dram_parameter("pswp", [128, 128], DT, isOutput=False)
    ident = nc.declare_dram_parameter("ident", [64, 64], DT, isOutput=False)
    mask0 = nc.declare_dram_parameter("mask0", [128, 256], DT, isOutput=False)
    mask1 = nc.declare_dram_parameter("mask1", [128, 256], DT, isOutput=False)
    outp = nc.declare_dram_parameter("out", [256, BT], F32, isOutput=True)

    with tile.TileContext(nc) as tc:
        with tc.tile_pool(name="persist", bufs=1) as persist, \
             tc.tile_pool(name="dram", bufs=1, space="DRAM") as dramp:
            qT_sb = persist.tile([64, HPC, BT], DT)
            kT_sb = persist.tile([64, BT], DT)
            v_sb = persist.tile([128, NTT, 65], DT)
            yloc = persist.tile([64, HPC, BT], DT)
            wo_sb = persist.tile([128, 16, 256], DT)
            pswp_sb = persist.tile([128, 128], DT)
            id_sb = persist.tile([64, 64], DT)
            m0_sb = persist.tile([128, 256], DT)
            m1_sb = persist.tile([128, 256], DT)

            ytloc = [dramp.tile([256, 512], DT, name=f"ytloc{c}")
                     for c in range(NCH)]
            ytful = [dramp.tile([2048, 512], DT, addr_space="Shared",
                                name=f"ytful{c}") for c in range(NCH)]

            nc.sync.dma_start(out=pswp_sb, in_=pswp[:])
            nc.sync.dma_start(out=id_sb, in_=ident[:])
            nc.sync.dma_start(out=m0_sb, in_=mask0[:])
            nc.sync.dma_start(out=m1_sb, in_=mask1[:])
            nc.vector.memset(v_sb[:, :, 64:65], 1.0)

            # ---------------- phase 1: projections + RoPE ----------------
            with tc.tile_pool(name="wpool", bufs=1) as wpool, \
                 tc.tile_pool(name="xs", bufs=2) as xs, \
                 tc.tile_pool(name="cs", bufs=2) as cs, \
                 tc.tile_pool(name="ptmp", bufs=3) as ptmp, \
                 tc.tile_pool(name="ps_mm", bufs=2, space="PSUM") as ps_mm, \
                 tc.tile_pool(name="ps_sw", bufs=2, space="PSUM") as ps_sw, \
                 tc.tile_pool(name="ps_vt", bufs=2, space="PSUM") as ps_vt:
                wq_sb = wpool.tile([128, 16, 256], DT)
                wkv_sb = wpool.tile([128, 16, 128], DT)
                nc.sync.dma_start(
                    out=wq_sb, in_=wqT.rearrange("(a p) o -> p a o", p=128))
                nc.sync.dma_start(
                    out=wkv_sb, in_=wkvT.rearrange("(a p) o -> p a o", p=128))

                for gg in range(NGG):
                    c0 = 1024 * gg
                    xt = xs.tile([128, 16, 1024], DT, tag="xt")
                    nc.sync.dma_start(
                        out=xt,
                        in_=xT[:, c0:c0 + 1024].rearrange(
                            "(a p) t -> p a t", p=128))
                    cost = cs.tile([128, 1024], DT, tag="cost")
                    sint = cs.tile([128, 1024], DT, tag="sint")
                    nc.sync.dma_start(out=cost, in_=cos2[:, c0:c0 + 1024])
                    nc.sync.dma_start(out=sint, in_=sin2[:, c0:c0 + 1024])

                    # m = 0,1: q head-pairs; m = 2: kv
                    for m in range(3):
                        pmm = ps_mm.tile([128, 2, 512], F32, tag="mm")
                        for ct in range(16):
                            if m < 2:
                                w_ap = wq_sb[:, ct, 128 * m:128 * m + 128]
                            else:
                                w_ap = wkv_sb[:, ct, :]
                            for sub in range(2):
                                nc.tensor.matmul(
                                    pmm[:, sub, :], lhsT=w_ap,
                                    rhs=xt[:, ct, 512 * sub:512 * sub + 512],
                                    start=(ct == 0), stop=(ct == 15))
                        for sub in range(2):
                            cols = slice(c0 + 512 * sub, c0 + 512 * sub + 512)
                            lc = slice(512 * sub, 512 * sub + 512)
                            if m < 2:
                                qraw = ptmp.tile([128, 512], DT, tag="qraw")
                                nc.scalar.copy(qraw, pmm[:, sub, :])
                                psw = ps_sw.tile([128, 512], F32, tag="sw")
                                nc.tensor.matmul(psw, lhsT=pswp_sb, rhs=qraw,
                                                 start=True, stop=True)
                                qsw = ptmp.tile([128, 512], DT, tag="qsw")
                                nc.scalar.copy(qsw, psw)
                                for hh in range(2):
                                    h = 2 * m + hh
                                    rows = slice(64 * hh, 64 * hh + 64)
                                    t0 = ptmp.tile([64, 512], DT, tag="t0")
                                    t1 = ptmp.tile([64, 512], DT, tag="t1")
                                    nc.vector.tensor_mul(
                                        t0, qraw[rows, :], cost[rows, lc])
                                    nc.vector.tensor_mul(
                                        t1, qsw[rows, :], sint[rows, lc])
                                    nc.vector.tensor_add(
                                        qT_sb[:, h, cols], t0, t1)
                            else:
                                kraw = ptmp.tile([64, 512], DT, tag="kraw")
                                vraw = ptmp.tile([64, 512], DT, tag="vraw")
                                nc.scalar.copy(kraw, pmm[0:64, sub, :])
                                nc.scalar.copy(vraw, pmm[64:128, sub, :])
                                psw = ps_sw.tile([128, 512], F32, tag="sw")
                                nc.tensor.matmul(
                                    psw[0:64, :], lhsT=pswp_sb[0:64, 0:64],
                                    rhs=kraw, start=True, stop=True)
                                ksw = ptmp.tile([64, 512], DT, tag="ksw")
                                nc.scalar.copy(ksw, psw[0:64, :])
                                t0 = ptmp.tile([64, 512], DT, tag="t0")
                                t1 = ptmp.tile([64, 512], DT, tag="t1")
                                nc.vector.tensor_mul(
                                    t0, kraw, cost[0:64, lc])
                                nc.vector.tensor_mul(
                                    t1, ksw, sint[0:64, lc])
                                nc.vector.tensor_add(kT_sb[:, cols], t0, t1)
                                for t4 in range(4):
                                    pvt = ps_vt.tile([128, 64], DT, tag="vt")
                                    nc.tensor.transpose(
                                        pvt, vraw[:, 128 * t4:128 * t4 + 128],
                                        id_sb)
                                    TT = (c0 + 512 * sub) // 128 + t4
                                    nc.vector.tensor_copy(
                                        v_sb[:, TT, 0:64], pvt)

            # ----------- phase 2: attention + gather + out-proj -----------
            with tc.tile_pool(name="ps_att", bufs=2, space="PSUM") as ps_att, \
                 tc.tile_pool(name="ps_pv", bufs=1, space="PSUM") as ps_pv, \
                 tc.tile_pool(name="ps_out", bufs=2, space="PSUM") as ps_out, \
                 tc.tile_pool(name="ptp", bufs=3) as ptp, \
                 tc.tile_pool(name="normp", bufs=4) as normp, \
                 tc.tile_pool(name="ytfp", bufs=2) as ytfp, \
                 tc.tile_pool(name="osbp", bufs=2) as osbp:
                nc.sync.dma_start(
                    out=wo_sb, in_=woT.rearrange("(a p) o -> p a o", p=128))

                def attention(ch):
                    for qbl in range(2):
                        qb = 2 * ch + qbl
                        b, p = qb // PQB, qb % PQB
                        qcols = slice(256 * qb, 256 * qb + 256)
                        nk = 2 * (p + 1)
                        po = ps_pv.tile([65, 4, 256], F32, tag="pv")
                        for kt in range(nk):
                            kc = slice(b * Tt + 128 * kt,
                                       b * Tt + 128 * kt + 128)
                            ktg = b * (Tt // 128) + kt
                            sm = ps_att.tile([128, 4, 256], F32, tag="smega")
                            for hp in range(2):
                                nc.tensor.matmul(
                                    sm[:, 2 * hp:2 * hp + 2, :],
                                    lhsT=kT_sb[:, kc],
                                    rhs=qT_sb[:, 2 * hp:2 * hp + 2, qcols],
                                    start=True, stop=True)
                            pt = ptp.tile([128, 4, 256], DT, tag="pt")
                            nc.scalar.activation(
                                pt, sm, mybir.ActivationFunctionType.Exp,
                                scale=0.125)
                            msk = None
                            if kt == nk - 2:
                                msk = m0_sb
                            elif kt == nk - 1:
                                msk = m1_sb
                            if msk is not None:
                                for h in range(4):
                                    nc.vector.tensor_mul(
                                        pt[:, h, :], pt[:, h, :], msk)
                            for hp in range(2):
                                nc.tensor.matmul(
                                    po[:, 2 * hp:2 * hp + 2, :],
                                    lhsT=v_sb[:, ktg, :],
                                    rhs=pt[:, 2 * hp:2 * hp + 2, :],
                                    start=(kt == 0), stop=(kt == nk - 1))
                        for h in range(4):
                            ssb = normp.tile([1, 256], F32, tag="ssb")
                            nc.vector.tensor_copy(ssb, po[64:65, h, :])
                            bca = normp.tile([64, 256], F32, tag="bca")
                            nc.gpsimd.partition_broadcast(bca, ssb)
                            rec = normp.tile([64, 256], F32, tag="rec")
                            nc.vector.reciprocal_approx_fast(rec, bca)
                            nc.vector.tensor_mul(yloc[:, h, qcols],
                                                 po[0:64, h, :], rec)
                    ccols = slice(512 * ch, 512 * ch + 512)
                    nc.sync.dma_start(
                        out=ytloc[ch].rearrange("(h d) t -> d h t", h=HPC),
                        in_=yloc[:, :, ccols])
                    nc.gpsimd.collective_compute(
                        "AllGather", mybir.AluOpType.bypass,
                        replica_groups=[list(range(NCORES))],
                        ins=[ytloc[ch]], outs=[ytful[ch]])

                def outproj(ch):
                    ytf = ytfp.tile([128, 16, 512], DT, tag="ytf")
                    nc.sync.dma_start(
                        out=ytf,
                        in_=ytful[ch].rearrange("(a p) t -> p a t", p=128))
                    for ot in range(2):
                        pout = ps_out.tile([128, 512], F32, tag="out")
                        for dt_ in range(16):
                            nc.tensor.matmul(
                                pout,
                                lhsT=wo_sb[:, dt_, 128 * ot:128 * ot + 128],
                                rhs=ytf[:, dt_, :],
                                start=(dt_ == 0), stop=(dt_ == 15))
                        ot_sb = osbp.tile([128, 512], F32, tag="osb")
                        nc.vector.tensor_copy(ot_sb, pout)
                        nc.sync.dma_start(
                            out=outp[128 * ot:128 * ot + 128,
                                     512 * ch:512 * ch + 512],
                            in_=ot_sb)

                for ch in range(NCH):
                    attention(ch)
                    if ch >= 1:
                        outproj(ch - 1)
                outproj(NCH - 1)

    nc.compile()
    return nc


def host_inputs(x, cos, sin, wq, wk, wv, wo, Tt=T):
    """Build the 8 per-core input maps from full fp32 inputs."""
    BT = B * Tt
    x = np.asarray(x, np.float32)[:, :Tt, :]
    xT = np.ascontiguousarray(x.reshape(BT, DIM).T).astype(BF)

    cos = np.asarray(cos, np.float32)[:Tt]
    sin = np.asarray(sin, np.float32)[:Tt]
    cos2 = np.empty((128, BT), np.float32)
    sin2 = np.empty((128, BT), np.float32)
    for d in range(128):
        j = (d % 64) // 2
        cos2[d] = np.tile(cos[:, j], B)
        sin2[d] = np.tile(sin[:, j] if d % 2 else -sin[:, j], B)

    pswp = np.zeros((128, 128), BF)
    for i in range(128):
        pswp[i, i ^ 1] = 1
    ident = np.eye(64, dtype=BF)
    ii = np.arange(128)[:, None]
    jj = np.arange(256)[None, :]
    mask0 = (jj >= ii).astype(BF)
    mask1 = (jj >= ii + 128).astype(BF)

    maps = []
    for c in range(NCORES):
        qs = slice(256 * c, 256 * c + 256)
        ks = slice(64 * c, 64 * c + 64)
        wkv = np.concatenate([wk[ks], wv[ks]], axis=0)
        maps.append({
            "xT": xT,
            "wqT": np.ascontiguousarray(wq[qs].T).astype(BF),
            "wkvT": np.ascontiguousarray(wkv.T).astype(BF),
            "woT": np.ascontiguousarray(wo[qs].T).astype(BF),
            "cos2": cos2.astype(BF), "sin2": sin2.astype(BF),
            "pswp": pswp, "ident": ident,
            "mask0": mask0, "mask1": mask1,
        })
    return maps


_NC_CACHE = {}


def _get_nc(Tt=T):
    if Tt not in _NC_CACHE:
        _NC_CACHE[Tt] = build_nc(Tt)
    return _NC_CACHE[Tt]


def kernel(x, cos, sin, wq, wk, wv, wo):
    global LAST_RESULTS
    nc = _get_nc(T)
    maps = host_inputs(x, cos, sin, wq, wk, wv, wo)
    res = run_bass_kernel_spmd(nc, maps, core_ids=list(range(NCORES)))
    LAST_RESULTS = res
    out = np.empty((B * T, DIM), np.float32)
    for c in range(NCORES):
        out[:, 256 * c:256 * c + 256] = res.results[c]["out"].T
    return out.reshape(B, T, DIM)
